# revision 2
# baseline (speedup 1.0000x reference)
"""CrossAttention kernel v3 for Trainium2 (8 NeuronCores, SPMD).

Reference math (B=4, C=256, N=4096, OUT=256, TEMP=sqrt(OUT)=16):
    q = Wq @ x; k = Wk @ xx; v = Wv @ xx
    attn = softmax(q^T k / TEMP, axis=-1)   (B, N, N)
    y = einsum('bnm,bom->bon', attn, v)     (B, OUT, N)

Sharding: 8 cores = (batch b, query-half h); each core computes its 2048
query rows against the full 4096 keys of its batch.

v3 structure:
    A = (TEMP*Wq^T Wk) folds the q/k projections (host, bf16, C x C);
    logits come out at 256x scale, exp() applies 1/256.
    u  = A^T-contracted x (bf16 matmul, fp32 PSUM) -> u8hi + u8lo fp8 pair
         (DVE eviction: hi = fp8(u), lo = fp8(u - hi))
    v  = xx^T Wv^T (bf16 matmul) -> v8hi + v8lo fp8 pair at 16x scale
         (v ~ N(0,0.33); x16 keeps the residual out of fp8 denormals);
         augmented column: 16.0 in v8hi, 0.0 in v8lo -> denominator.
    S2 logits: S^T = xkv8^T u8hi + xkv8^T u8lo   (2 fp8 DoubleRow matmuls,
         256-channel contraction; K-side single-fp8 costs ~0.6% error
         because per-key quant noise averages out over 4096 keys).
    S1 logits (optional per block): single matmul with u8hi only.
    P8 = exp(S/256) written directly as fp8e4 by the Activation engine;
         optionally `dve_exp_pairs` m-tile pairs per block use a custom
         DVE op EXP44 (e^z = (poly3(z/4))^4, rel err < 0.5%) to split the
         exp work across ACT + DVE.
    PV: y^T = P8-tiles^T (v8hi + v8lo)  (2 fp8 DR matmuls per 256-key
         group, FD=257; last column = 16*denominator, cancels exactly).
    y = y^T[:, :256] / y^T[:, 256]  -> bf16, host does final transpose.
"""

import numpy as np
import ml_dtypes
from contextlib import ExitStack

import concourse.bass as bass
import concourse.tile as tile
from concourse import bacc, mybir
from concourse.bass_utils import run_bass_kernel_spmd

B, C, NSEQ, OUT = 4, 256, 4096, 256
TEMP = float(OUT) ** 0.5
NCORES = 8
BF16 = mybir.dt.bfloat16
F32 = mybir.dt.float32
FP8 = mybir.dt.float8e4
BFNP = ml_dtypes.bfloat16
F8NP = ml_dtypes.float8_e4m3
DR = mybir.MatmulPerfMode.DoubleRow
EXP = mybir.ActivationFunctionType.Exp
ESCALE = 1.0 / 256.0
VSCALE = 16.0

# ---- custom DVE op: EXP44(x) = (poly3(x))^4 ~= exp(x/256) ----------------
_EXP44_C = (0.9989470352890082, 0.0009792188753603966,
            4.957915933995179e-07, 1.5141082481223778e-10)
_EXP44 = None


def _get_exp44():
    """Register the EXP44 custom DVE op (idempotent)."""
    global _EXP44
    if _EXP44 is not None:
        return _EXP44
    import concourse.dve_ops as dve_ops
    from concourse.dve_spec import (Spec, Src0, C0, C1, C2, sq, lower,
                                    _spill_c3_to_src1)
    from concourse.dve_spec import _has_src1 as has_src1
    from concourse.dve_spec import C3
    from concourse.dve_uop import DveOpSpec

    for op in dve_ops.OPS:
        if op.name == "EXP44_ANT":
            _EXP44 = op
            return op
    body = sq(sq(((Src0 * C3 + C2) * Src0 + C1) * Src0 + C0))
    spec = Spec(body=_spill_c3_to_src1(body))
    shas = {}
    for ver in ("v3", "v4"):
        try:
            s = DveOpSpec(name="EXP44_ANT", opcode=0,
                          uops=lower(spec, ver=ver), rd1_en=has_src1(spec))
            shas[ver] = s.sha(ver)
        except Exception:
            pass
    op = dve_ops.DveOp("EXP44_ANT", spec, subdim=False, uops_sha=shas)
    dve_ops.OPS.append(op)
    dve_ops.CUSTOM_DVE_SPECS[op.name] = op.spec
    dve_ops._SUB_OPCODE_FOR_NAME[op.name] = (
        dve_ops._CUSTOM_DVE_ROW_BASE + len(dve_ops.OPS) - 1)
    _EXP44 = op
    return op


def build(bc=2048, m=4096, nblk=512, repeat_full=1,
          s_modes=("S2", "S2", "S2", "S2"), pv_mode="P2", dve_exp_pairs=0):
    """Build the per-core SPMD Bass program.

    bc: query rows per core; m: key count; nblk: query block width.
    s_modes: per query-block logit matmul mode, "S2" (K8 x (u8hi+u8lo))
    or "S1" (K8 x u8hi only). pv_mode: "P2" (fp8 DR PV) or "bf16".
    dve_exp_pairs: of the 16 m-tile pairs per block, how many run exp on
    the Vector engine (EXP44) instead of the Activation engine.
    """
    ct = C // 128     # contraction tiles over channels
    mt = m // 128     # key tiles
    nb = bc // nblk   # query blocks
    nt = nblk // 128  # 128-query tiles per block
    qch = bc // 512
    kch = m // 512
    assert len(s_modes) == nb
    if dve_exp_pairs:
        exp44 = _get_exp44()
    PDT = FP8 if pv_mode == "P2" else BF16

    nc = bacc.Bacc("TRN2", target_bir_lowering=False, debug=False,
                   num_devices=NCORES)
    x_d = nc.dram_tensor("xq", [ct, 128, bc], BF16, kind="ExternalInput")
    xkv_d = nc.dram_tensor("xkv", [ct, 128, m], BF16, kind="ExternalInput")
    a_d = nc.dram_tensor("aT", [ct, 128, C], BF16, kind="ExternalInput")
    wv_d = nc.dram_tensor("wvT", [ct, 128, OUT], BF16, kind="ExternalInput")
    xkv8_d = nc.dram_tensor("xkv8", [ct, 128, m], FP8, kind="ExternalInput")
    y_d = nc.dram_tensor("y", [nb, 128, nt * OUT], BF16,
                         kind="ExternalOutput")

    with tile.TileContext(nc) as tc, ExitStack() as ctx:
        const = ctx.enter_context(tc.tile_pool(name="const", bufs=1))
        dbl = ctx.enter_context(tc.tile_pool(name="dbl", bufs=2))

        x_sb = const.tile([128, ct, bc], BF16, name="x_sb")
        a_sb = const.tile([128, ct, C], BF16, name="a_sb")
        wv_sb = const.tile([128, ct, OUT], BF16, name="wv_sb")
        zbias = const.tile([128, 1], F32, name="zbias")
        nc.vector.memset(zbias[:], 0.0)
        if dve_exp_pairs:
            c3_sb = const.tile([128, 1], F32, name="c3_sb")
            nc.vector.memset(c3_sb[:], _EXP44_C[3])

        for _rf in range(repeat_full):
            xkv_sb = dbl.tile([128, ct, m], BF16, tag="xkv", name="xkv_sb")
            xkv8_sb = dbl.tile([128, ct, m], FP8, tag="xkv8", name="xkv8_sb")
            u8hi_sb = dbl.tile([128, ct, bc], FP8, tag="u8hi", name="u8hi_sb")
            u8lo_sb = dbl.tile([128, ct, bc], FP8, tag="u8lo", name="u8lo_sb")
            if pv_mode == "P2":
                # padded to a 272-byte m-tile stride: the DoubleRow moving
                # operand requires the k-pair step to be 16-byte aligned
                v8hi_sb = dbl.tile([128, mt, OUT + 16], FP8, tag="v8hi",
                                   name="v8hi_sb")
                v8lo_sb = dbl.tile([128, mt, OUT + 16], FP8, tag="v8lo",
                                   name="v8lo_sb")
            else:
                v16_sb = dbl.tile([128, mt, OUT + 1], BF16, tag="v16",
                                  name="v16_sb")
            y_sb = dbl.tile([128, nb, nt * OUT], BF16, tag="y", name="y_sb")

            # DMA order follows consumption: A + x (u projection) first,
            # then xkv8 (S matmuls), then wv + xkv (v projection). One
            # dma_start per tensor — the double-buffered pool prefetches
            # during the previous repeat, so chunking buys nothing and
            # per-DMA DGE setup cost dominates with many small copies.
            for i in range(ct):
                nc.sync.dma_start(a_sb[:, i, :], a_d.ap()[i])
            nc.sync.dma_start(x_sb[:], x_d.ap().transpose([1, 0, 2]))
            for i in range(ct):
                nc.sync.dma_start(wv_sb[:, i, :], wv_d.ap()[i])
            nc.sync.dma_start(xkv8_sb[:], xkv8_d.ap().transpose([1, 0, 2]))
            nc.sync.dma_start(xkv_sb[:], xkv_d.ap().transpose([1, 0, 2]))
            if pv_mode == "P2":
                nc.vector.memset(v8hi_sb[:, :, OUT:OUT + 1], VSCALE)
                nc.vector.memset(v8lo_sb[:, :, OUT:OUT + 1], 0.0)
            else:
                nc.vector.memset(v16_sb[:, :, OUT:OUT + 1], 1.0)

            # ---- u projection (bf16 matmuls, fp8 hi/lo DVE evictions) ----
            # u first, chunk-major: the S stage of query-block 0 only needs
            # u8hi/lo[:, :, 0:512], so its evictions land first and the
            # attention matmuls start immediately after. The v projection is
            # interleaved into the attention loop below (it borrows s-pool
            # PSUM tiles and its evictions run on the idle Pool engine) so
            # it hides under the S matmuls of blocks 0-1.
            with tc.tile_pool(name="u_ps", bufs=3, space="PSUM") as u_pool:
                for chk in range(qch):
                    for co in range(ct):
                        ps = u_pool.tile([128, 512], F32, tag="u", name="u_t")
                        for c in range(ct):
                            nc.tensor.matmul(
                                ps[:], a_sb[:, c, co * 128:(co + 1) * 128],
                                x_sb[:, c, chk * 512:(chk + 1) * 512],
                                start=(c == 0), stop=(c == ct - 1))
                        sl = slice(chk * 512, (chk + 1) * 512)
                        nc.vector.tensor_copy(u8hi_sb[:, co, sl], ps[:])
                        nc.vector.tensor_sub(u8lo_sb[:, co, sl], ps[:],
                                             u8hi_sb[:, co, sl])

            # ---- attention ----
            with tc.tile_pool(name="p_sb", bufs=2) as p_pool, \
                 tc.tile_pool(name="s_ps", bufs=3, space="PSUM") as s_pool, \
                 tc.tile_pool(name="y_ps", bufs=2, space="PSUM") as y_pool, \
                 tc.tile_pool(name="fin", bufs=8) as fin_pool:
                def v_proj_group(g):
                    """Project v m-tiles 16g..16g+15 into s-pool PSUM
                    tiles (2 m-tiles per [128,2,512] tile, one per bank so
                    the matmul target stays bank-aligned), evicting to
                    v8hi/v8lo fp8 (or v16 bf16) on the Pool engine."""
                    for t in range(8):
                        ps = s_pool.tile([128, 2, nblk], F32, tag="s",
                                         name="vps_t")
                        for q in range(2):
                            mi = 16 * g + 2 * t + q
                            reg = ps[:, q, 0:OUT]
                            for c in range(ct):
                                nc.tensor.matmul(
                                    reg, xkv_sb[:, c, mi * 128:(mi + 1) * 128],
                                    wv_sb[:, c, :],
                                    start=(c == 0), stop=(c == ct - 1))
                        for q in range(2):
                            mi = 16 * g + 2 * t + q
                            reg = ps[:, q, 0:OUT]
                            if pv_mode == "P2":
                                nc.vector.tensor_scalar_mul(
                                    v8hi_sb[:, mi, 0:OUT], reg, VSCALE)
                                nc.vector.scalar_tensor_tensor(
                                    v8lo_sb[:, mi, 0:OUT], reg, VSCALE,
                                    v8hi_sb[:, mi, 0:OUT],
                                    mybir.AluOpType.mult,
                                    mybir.AluOpType.subtract)
                            else:
                                nc.scalar.copy(v16_sb[:, mi, 0:OUT], reg)

                P_tiles = [None] * nb
                for blk in range(nb + 1):
                    if blk < nb:
                        n0 = blk * nblk
                        P_sb = p_pool.tile([128, mt, nblk], PDT, tag="p",
                                           name="P_sb")
                        P_tiles[blk] = P_sb
                        for mj in range(mt // 2):
                            s_ps = s_pool.tile([128, 2, nblk], F32, tag="s",
                                               name="s_t")
                            for half in range(2):
                                mi = 2 * mj + half
                                if s_modes[blk] == "S1":
                                    nc.tensor.matmul(
                                        s_ps[:, half, :],
                                        xkv8_sb[:, :, mi * 128:(mi + 1) * 128],
                                        u8hi_sb[:, :, n0:n0 + nblk],
                                        start=True, stop=True, perf_mode=DR)
                                else:
                                    nc.tensor.matmul(
                                        s_ps[:, half, :],
                                        xkv8_sb[:, :, mi * 128:(mi + 1) * 128],
                                        u8hi_sb[:, :, n0:n0 + nblk],
                                        start=True, stop=False, perf_mode=DR)
                                    nc.tensor.matmul(
                                        s_ps[:, half, :],
                                        xkv8_sb[:, :, mi * 128:(mi + 1) * 128],
                                        u8lo_sb[:, :, n0:n0 + nblk],
                                        start=False, stop=True, perf_mode=DR)
                            if mj >= mt // 2 - dve_exp_pairs:
                                nc.vector._custom_dve(
                                    exp44,
                                    out=P_sb[:, 2 * mj:2 * mj + 2, :],
                                    in0=s_ps[:], in1=c3_sb[:],
                                    s0=_EXP44_C[0], s1=_EXP44_C[1],
                                    imm2=_EXP44_C[2])
                            else:
                                nc.scalar.activation(
                                    P_sb[:, 2 * mj:2 * mj + 2, :], s_ps[:],
                                    EXP, bias=zbias[:], scale=ESCALE)
                    if blk < 2:
                        v_proj_group(blk)
                    if blk == 0:
                        continue
                    P_sb = P_tiles[blk - 1]
                    for ni in range(nt):
                        y_ps = y_pool.tile([128, OUT + 1], F32, tag="y",
                                           name="y_t")
                        if pv_mode == "P2":
                            for mj in range(mt // 2):
                                nc.tensor.matmul(
                                    y_ps[:],
                                    P_sb[:, 2 * mj:2 * mj + 2,
                                         ni * 128:(ni + 1) * 128],
                                    v8hi_sb[:, 2 * mj:2 * mj + 2, 0:OUT + 1],
                                    start=(mj == 0), stop=False,
                                    perf_mode=DR)
                                nc.tensor.matmul(
                                    y_ps[:],
                                    P_sb[:, 2 * mj:2 * mj + 2,
                                         ni * 128:(ni + 1) * 128],
                                    v8lo_sb[:, 2 * mj:2 * mj + 2, 0:OUT + 1],
                                    start=False, stop=(mj == mt // 2 - 1),
                                    perf_mode=DR)
                        else:
                            for mi in range(mt):
                                nc.tensor.matmul(
                                    y_ps[:],
                                    P_sb[:, mi, ni * 128:(ni + 1) * 128],
                                    v16_sb[:, mi, :],
                                    start=(mi == 0), stop=(mi == mt - 1))
                        recip = fin_pool.tile([128, 1], F32, tag="recip",
                                              name="recip")
                        nc.vector.reciprocal(recip[:], y_ps[:, OUT:OUT + 1])
                        nc.vector.tensor_scalar_mul(
                            y_sb[:, blk - 1, ni * OUT:(ni + 1) * OUT],
                            y_ps[:, 0:OUT], recip[:])
                    nc.sync.dma_start(y_d.ap()[blk - 1],
                                      y_sb[:, blk - 1, :])
    nc.compile()
    return nc


def make_in_maps(x, xx, Wq, Wk, Wv, bc=2048, m=4096):
    """Host-side prep: slice/cast per-core inputs. Returns list of 8 dicts."""
    ct = C // 128
    A = (TEMP * (np.asarray(Wq).T @ np.asarray(Wk))).astype(BFNP)  # (C, C)
    a_t = np.ascontiguousarray(A.reshape(ct, 128, C))
    wv_t = np.ascontiguousarray(np.asarray(Wv).T.astype(BFNP)
                                .reshape(ct, 128, OUT))
    halves = NCORES // B
    in_maps = []
    for core in range(NCORES):
        b, h = divmod(core, halves)
        xq = np.ascontiguousarray(
            x[b, :, h * bc:(h + 1) * bc].astype(BFNP).reshape(ct, 128, bc))
        xkv = np.ascontiguousarray(
            xx[b, :, :m].astype(BFNP).reshape(ct, 128, m))
        xkv8 = np.ascontiguousarray(
            xx[b, :, :m].astype(F8NP).reshape(ct, 128, m))
        in_maps.append({"xq": xq, "xkv": xkv, "xkv8": xkv8, "aT": a_t,
                        "wvT": wv_t})
    return in_maps


def gather_output(results, bc=2048, nblk=512):
    """Reassemble per-core y^T outputs into (B, OUT, NSEQ)."""
    nb, nt = bc // nblk, nblk // 128
    y = np.empty((B, OUT, NSEQ), dtype=np.float32)
    halves = NCORES // B
    for core, res in enumerate(results):
        b, h = divmod(core, halves)
        yt = np.asarray(res["y"], dtype=np.float32)  # (nb, 128, nt*OUT)
        yt = yt.reshape(nb, 128, nt, OUT).transpose(0, 2, 1, 3)
        y[b, :, h * bc:(h + 1) * bc] = yt.reshape(bc, OUT).T
    return y


_NC_CACHE = {}


def kernel(x, xx, Wq, Wk, Wv):
    x = np.asarray(x)
    xx = np.asarray(xx)
    key = "full"
    if key not in _NC_CACHE:
        _NC_CACHE[key] = build()
    nc = _NC_CACHE[key]
    in_maps = make_in_maps(x, xx, np.asarray(Wq), np.asarray(Wk),
                           np.asarray(Wv))
    try:
        res = run_bass_kernel_spmd(nc, in_maps, core_ids=list(range(NCORES)))
    except Exception:
        res = run_bass_kernel_spmd(nc, in_maps, core_ids=list(range(NCORES)))
    return gather_output(res.results)
